# revision 5
# baseline (speedup 1.0000x reference)
"""Distributed causal attention head on 8 TRN2 NeuronCores (v2: parity split).

Problem: B=4, S=4096, D_in=512, D_out=64 causal attention
  K/V/Q = X @ W; scores = Q@K^T (causal, /sqrt(64)); Z = softmax(scores)@V

Sharding: core c = 2*b + p handles batch b and the KEY/VALUE blocks of
parity p (global 128-row k-blocks {2l+p}).  Each core computes partial
attention numerators Z_p^T = V_p^T P_p and partial denominators over its
k-parity for ALL q of the batch; the host sums the two partials per
batch and normalizes (exactly the flash-attention partial-softmax
combine, with no max-subtraction needed since |scores/8| < ~1.5).

Schedule (identical on all cores; parity differences live in the DATA:
xk/xv contents and a single [128,256] causal mask):
 - prologue: project K (parity half), Q (full), V (parity half) with
   one PSUM piece per 512 cols; PE-transpose V to k-major vp blocks
   with a ones-column appended (gives the denominator for free).
 - attention over q-chunks [0,2048),[2048,3072),[3072,4096): for each
   local k-block l the score matmuls stream ONLY the causal-valid
   q-suffix (offset 256*l, the max over both parities), exp runs on ACT
   in [128,<=1024] windows, the one partially-masked 256-col region is
   multiplied by the per-core mask on DVE, and AV matmuls accumulate
   zt[65, q] in PSUM with start/stop flags per 512-col PSUM bank.
   Finished 512-col strips are copied to SBUF and DMA'd out as soon as
   their last k-block has accumulated.
Matmul inputs bf16, PSUM f32, output partials f32.
"""

import numpy as np
import ml_dtypes

import concourse.bass as bass
import concourse.bacc as bacc
import concourse.mybir as mybir
import concourse.tile as tile

B, S, D, E = 4, 4096, 512, 64
PB = 128                     # partition block
NL = 16                      # local k-blocks per core (parity half)
ND = D // PB                 # 4 d-slices
LAG = 2                      # ST -> AV pipeline depth (in l's)
# attention q-chunks: (q0, width, Lmax). Lmax = last local k-block with
# causal work in the chunk (uniform over parity).
CHUNKS = [(0, 2048, 7), (2048, 1024, 11), (3072, 1024, 15)]
BF16 = mybir.dt.bfloat16
F32 = mybir.dt.float32
NPBF16 = ml_dtypes.bfloat16


def st_segs(o, W):
    """512-bank-aligned segments covering [o, W)."""
    segs, a = [], o
    while a < W:
        b = min((a // 512 + 1) * 512, W)
        segs.append((a, b))
        a = b
    return segs


def l_last(q0, s, Lmax):
    """Last local k-block whose stream covers 512-col strip s of chunk."""
    return min(Lmax, (q0 + 512 * s + 511) // 256)


def build_nc():
    nc = bacc.Bacc(None)

    xq_d = nc.declare_dram_parameter("xq", [D, S], BF16, isOutput=False)
    xk_d = nc.declare_dram_parameter("xk", [D, S // 2], BF16, isOutput=False)
    xv_d = nc.declare_dram_parameter("xv", [D, S // 2], BF16, isOutput=False)
    wq_d = nc.declare_dram_parameter("wq", [D, E], BF16, isOutput=False)
    wk_d = nc.declare_dram_parameter("wk", [D, E], BF16, isOutput=False)
    wv_d = nc.declare_dram_parameter("wv", [D, E], BF16, isOutput=False)
    cm_d = nc.declare_dram_parameter("cmask", [PB, 256], BF16, isOutput=False)
    id_d = nc.declare_dram_parameter("ident", [E, E], BF16, isOutput=False)
    out_d = nc.declare_dram_parameter("out", [E + 1, S], F32, isOutput=True)

    with tile.TileContext(nc) as tc:
        with tc.tile_pool(name="persist", bufs=1) as pp, \
             tc.tile_pool(name="work", bufs=6) as wp, \
             tc.tile_pool(name="osb", bufs=3) as op:
            # ---- persistent SBUF tiles ----
            wq_sb = pp.tile([PB, ND * E], BF16, name="wq_sb", tag="wq_sb")
            wk_sb = pp.tile([PB, ND * E], BF16, name="wk_sb", tag="wk_sb")
            wv_sb = pp.tile([PB, ND * E], BF16, name="wv_sb", tag="wv_sb")
            mk_sb = pp.tile([PB, 256], BF16, name="mk_sb", tag="mk_sb")
            idb_sb = pp.tile([E, E], BF16, name="idb_sb", tag="idb_sb")
            xq_sb = [[pp.tile([PB, 2048], BF16, name=f"xq{d}_{h}", tag=f"xq{d}_{h}")
                      for h in range(2)] for d in range(ND)]
            xk_sb = [pp.tile([PB, 2048], BF16, name=f"xk{d}", tag=f"xk{d}")
                     for d in range(ND)]
            xv_sb = [pp.tile([PB, 2048], BF16, name=f"xv{d}", tag=f"xv{d}")
                     for d in range(ND)]
            qpT = pp.tile([E, S], BF16, name="qpT", tag="qpT")
            kpT = pp.tile([E, S // 2], BF16, name="kpT", tag="kpT")
            vpT = pp.tile([E, S // 2], BF16, name="vpT", tag="vpT")
            vp = pp.tile([PB, NL * (E + 1)], BF16, name="vp", tag="vp")

            # ---- DMAs (two queues: sync + gpsimd) ----
            # gpsimd queue: weights/mask/ident first, then its input share
            for w_d, w_sb in ((wk_d, wk_sb), (wq_d, wq_sb), (wv_d, wv_sb)):
                nc.gpsimd.dma_start(
                    out=w_sb[:].rearrange("p (d e) -> p d e", e=E),
                    in_=w_d.rearrange("(d p) e -> p d e", p=PB))
            nc.gpsimd.dma_start(out=mk_sb[:], in_=cm_d[:])
            nc.gpsimd.dma_start(out=idb_sb[:], in_=id_d[:])
            for d in (2, 3):
                nc.gpsimd.dma_start(out=xk_sb[d][:], in_=xk_d[PB * d:PB * (d + 1), :])
            # sync queue: xk d0/d1 first (K proj is first PE work)
            for d in (0, 1):
                nc.sync.dma_start(out=xk_sb[d][:], in_=xk_d[PB * d:PB * (d + 1), :])
            # xq half 0 split across queues, then xv, then xq half 1
            for d in (0, 1):
                nc.gpsimd.dma_start(out=xq_sb[d][0][:],
                                    in_=xq_d[PB * d:PB * (d + 1), 0:2048])
            for d in (2, 3):
                nc.sync.dma_start(out=xq_sb[d][0][:],
                                  in_=xq_d[PB * d:PB * (d + 1), 0:2048])
            for d in range(ND):
                nc.sync.dma_start(out=xv_sb[d][:], in_=xv_d[PB * d:PB * (d + 1), :])
            for d in (0, 1):
                nc.gpsimd.dma_start(out=xq_sb[d][1][:],
                                    in_=xq_d[PB * d:PB * (d + 1), 2048:4096])
            for d in (2, 3):
                nc.sync.dma_start(out=xq_sb[d][1][:],
                                  in_=xq_d[PB * d:PB * (d + 1), 2048:4096])

            # ones column of vp (written once; disjoint from transpose copies)
            nc.vector.memset(
                vp[:].rearrange("p (l e) -> p l e", e=E + 1)[:, :, E:E + 1], 1.0)

            # ---- prologue: projections (PSUM pj pieces of 512 cols) ----
            with tc.tile_pool(name="pj_ps", bufs=4, space="PSUM") as pjp, \
                 tc.tile_pool(name="vt_ps", bufs=2, space="PSUM") as vtp:
                def proj_piece(w_sb, rhs_tile, rhs_off, dst, dst_off):
                    pj = pjp.tile([E, 512], F32, name=f"pj{dst_off}", tag="pj")
                    for d in range(ND):
                        nc.tensor.matmul(pj[:], w_sb[:, E * d:E * (d + 1)],
                                         rhs_tile[d][:, rhs_off:rhs_off + 512],
                                         start=(d == 0), stop=(d == ND - 1))
                    nc.vector.tensor_copy(dst[:, dst_off:dst_off + 512], pj[:])

                for i in range(4):      # K: parity half, 2048 cols
                    proj_piece(wk_sb, xk_sb, 512 * i, kpT, 512 * i)
                for j in range(4):      # Q half 0
                    proj_piece(wq_sb, [xq_sb[d][0] for d in range(ND)],
                               512 * j, qpT, 512 * j)
                for i in range(4):      # V: parity half
                    proj_piece(wv_sb, xv_sb, 512 * i, vpT, 512 * i)
                # V -> k-major vp blocks (PE transpose, batched via PSUM)
                for t in range(2):
                    vt = vtp.tile([PB, 8 * E], BF16, name=f"vt{t}", tag="vt")
                    for j in range(8):
                        l = 8 * t + j
                        nc.tensor.transpose(vt[:, E * j:E * (j + 1)],
                                            vpT[:, PB * l:PB * (l + 1)],
                                            idb_sb[:])
                    nc.vector.tensor_copy(
                        vp[:].rearrange("p (l e) -> p l e", e=E + 1)[:, 8 * t:8 * t + 8, 0:E],
                        vt[:].rearrange("p (l e) -> p l e", e=E))
                for j in range(4, 8):   # Q half 1
                    proj_piece(wq_sb, [xq_sb[d][1] for d in range(ND)],
                               512 * (j - 4), qpT, 512 * j)

            # ---- attention ----
            with tc.tile_pool(name="st_ps", bufs=2, space="PSUM") as stp, \
                 tc.tile_pool(name="zt_ps", bufs=1, space="PSUM") as ztp:
                for (q0, W, Lmax) in CHUNKS:
                    zt = ztp.tile([E + 1, 2048], F32, name=f"zt{q0}", tag="zt")
                    pend = []

                    def drain_av(l, o, ets, q0=q0, W=W, Lmax=Lmax, zt=zt):
                        for (a, b) in st_segs(o, W):
                            et, u = ets[a // 1024]
                            nc.tensor.matmul(
                                zt[:, a:b], vp[:, (E + 1) * l:(E + 1) * (l + 1)],
                                et[:, a - 1024 * u:b - 1024 * u],
                                start=(l == 0),
                                stop=(l == l_last(q0, a // 512, Lmax)),
                                skip_group_check=True)
                        # strips whose accumulation just completed -> out
                        for s in range(W // 512):
                            if l_last(q0, s, Lmax) == l:
                                zs = op.tile([E + 1, 512], F32, name=f"zs{q0}_{s}", tag="zs")
                                nc.vector.tensor_copy(zs[:], zt[:, 512 * s:512 * (s + 1)])
                                nc.gpsimd.dma_start(
                                    out=out_d[:, q0 + 512 * s:q0 + 512 * (s + 1)],
                                    in_=zs[:])

                    for l in range(Lmax + 1):
                        o = max(0, 256 * l - q0)
                        segs = st_segs(o, W)
                        # score matmuls (512-col PSUM-bank pieces)
                        st_tiles = {}
                        for u in sorted({a // 1024 for (a, _) in segs}):
                            st_tiles[u] = stp.tile([PB, 1024], F32, name=f"st{q0}_{l}_{u}", tag="st")
                        for (a, b) in segs:
                            u = a // 1024
                            nc.tensor.matmul(
                                st_tiles[u][:, a - 1024 * u:b - 1024 * u],
                                kpT[:, PB * l:PB * (l + 1)],
                                qpT[:, q0 + a:q0 + b],
                                start=True, stop=True)
                        # exp (+ causal mask on the one partial region)
                        ets = {}
                        for u, stt in st_tiles.items():
                            et = wp.tile([PB, 1024], BF16, name=f"et{q0}_{l}_{u}", tag="et")
                            aw = max(o - 1024 * u, 0)
                            ew = min(W - 1024 * u, 1024)
                            nc.scalar.activation(
                                et[:, aw:ew], stt[:, aw:ew],
                                mybir.ActivationFunctionType.Exp, scale=0.125)
                            ets[u] = (et, u)
                        if q0 <= 256 * l < q0 + W:   # diagonal in this chunk
                            u = o // 1024
                            mo = o - 1024 * u
                            et = ets[u][0]
                            nc.vector.tensor_mul(et[:, mo:mo + 256],
                                                 et[:, mo:mo + 256], mk_sb[:])
                        pend.append((l, o, ets))
                        if len(pend) > LAG:
                            drain_av(*pend.pop(0))
                    while pend:
                        drain_av(*pend.pop(0))
    nc.finalize()
    return nc


def make_core_inputs(key_np, value_np, query_np, Wk, Wv, Wq):
    """Host-side sharding: returns in_maps list of 8 dicts."""
    bf = lambda a: np.ascontiguousarray(a).astype(NPBF16)
    ki = np.arange(PB)[:, None]
    qi = np.arange(PB)[None, :]
    tri = (ki <= qi).astype(np.float32)
    ones = np.ones((PB, PB), np.float32)
    zeros = np.zeros((PB, PB), np.float32)
    in_maps = []
    for c in range(8):
        b, p = c // 2, c % 2
        kcols = np.concatenate(
            [np.arange(PB * (2 * l + p), PB * (2 * l + p) + PB) for l in range(NL)])
        cmask = np.concatenate([tri, ones] if p == 0 else [zeros, tri], axis=1)
        in_maps.append({
            "xq": bf(query_np[b].T),
            "xk": bf(key_np[b].T[:, kcols]),
            "xv": bf(value_np[b].T[:, kcols]),
            "wq": bf(Wq), "wk": bf(Wk), "wv": bf(Wv),
            "cmask": bf(cmask),
            "ident": bf(np.eye(E, dtype=np.float32)),
        })
    return in_maps


def assemble_output(results):
    """results: 8 dicts with 'out' [65, S] f32 partials -> Z [B,S,E]."""
    Z = np.zeros((B, S, E), dtype=np.float32)
    for b in range(B):
        A = results[2 * b]["out"].astype(np.float32) + \
            results[2 * b + 1]["out"].astype(np.float32)
        Z[b] = (A[:E] / A[E:E + 1]).T
    return Z


def kernel(key_inputs, value_inputs, query_inputs, Wk, Wv, Wq):
    from concourse.bass_utils import run_bass_kernel_spmd
    nc = build_nc()
    in_maps = make_core_inputs(np.asarray(key_inputs), np.asarray(value_inputs),
                               np.asarray(query_inputs), np.asarray(Wk),
                               np.asarray(Wv), np.asarray(Wq))
    res = run_bass_kernel_spmd(nc, in_maps, core_ids=list(range(8)))
    return assemble_output(res.results)


# revision 7
# speedup vs baseline: 1.0222x; 1.0222x over previous
"""Distributed causal attention head on 8 TRN2 NeuronCores.

v3: parity-split sharding + fp8 DoubleRow score path.

Sharding: core c = 2*b + p handles batch b and the KEY/VALUE blocks of
parity p (global 128-row k-blocks {2l+p}).  Each core computes partial
attention numerators Z_p^T = V_p^T P_p and partial denominators over its
k-parity for ALL q of the batch; the host sums the two partials per
batch and normalizes (flash-attention partial-softmax combine; no
max-subtraction needed since |scores/8| < ~1.5).

The Q/K path runs in fp8e4m3 with DoubleRow matmuls (2 contraction
subtiles per pass, 0.5 cycles/col): X_q/X_k arrive fp8 plane-packed,
W_q/W_k arrive fp8 pre-scaled by 32 (folded out in the exp scale), the
projections emit Q^T/K^T as [32, 2-plane] fp8, and score matmuls
contract E=64 as 2x32 planes.  The V/AV path stays bf16 for accuracy
(fp8 probs/values fail the 2e-2 gate).  PSUM is always f32.

Schedule is SPMD-identical across cores; parity lives in the DATA
(xk/xv contents and one [128,256] causal mask).  Score matmuls stream
only the causal-valid q-suffix (offset 256*l = max over parities); the
one partially-masked 256-col region per k-block is masked on DVE after
the ACT exp.  AV accumulates zt[65, q] in PSUM banks with start/stop
per 512-col bank; finished strips are copied out and DMA'd as soon as
their last k-block lands.  Projections run in a PSUM-pool prologue
(copies alternate DVE/ACT to keep pace with the PE).
"""

import numpy as np
import ml_dtypes

import concourse.bass as bass
import concourse.bacc as bacc
import concourse.mybir as mybir
import concourse.tile as tile

B, S, D, E = 4, 4096, 512, 64
PB = 128                     # partition block
NL = 16                      # local k-blocks per core (parity half)
ND = 4                       # d-slices
LAG = 2                      # ST -> AV pipeline depth (in l's)
WSC = 32.0                   # fp8 weight pre-scale (host); folded into exp
# attention q-chunks: (q0, width, Lmax)
CHUNKS = [(0, 2048, 7), (2048, 1024, 11), (3072, 1024, 15)]
BF16 = mybir.dt.bfloat16
F32 = mybir.dt.float32
F8 = mybir.dt.float8e4
NPBF16 = ml_dtypes.bfloat16
NPF8 = ml_dtypes.float8_e4m3
DR = mybir.MatmulPerfMode.DoubleRow


def st_segs(o, W):
    """512-bank-aligned segments covering [o, W)."""
    segs, a = [], o
    while a < W:
        b = min((a // 512 + 1) * 512, W)
        segs.append((a, b))
        a = b
    return segs


def l_last(q0, s, Lmax):
    """Last local k-block whose stream covers 512-col strip s of chunk."""
    return min(Lmax, (q0 + 512 * s + 511) // 256)


def build_nc():
    nc = bacc.Bacc(None)

    # fp8 Q/K inputs, plane-packed on host:
    #   xq8[p, (H,g,s,j)]: H = q-col half (2048), g = d-pair, s = d-slice in pair
    #   xk8[p, (g,s,j)]  : j over the 2048 parity-packed k cols
    #   wq8/wk8[p, (g,s,m,e)]: m = E-half (out plane), e in 0..31, pre-scaled x32
    xq8_d = nc.declare_dram_parameter("xq8", [PB, 16384], F8, isOutput=False)
    xk8_d = nc.declare_dram_parameter("xk8", [PB, 8192], F8, isOutput=False)
    wq8_d = nc.declare_dram_parameter("wq8", [PB, 256], F8, isOutput=False)
    wk8_d = nc.declare_dram_parameter("wk8", [PB, 256], F8, isOutput=False)
    xv_d = nc.declare_dram_parameter("xv", [D, S // 2], BF16, isOutput=False)
    wv_d = nc.declare_dram_parameter("wv", [D, E], BF16, isOutput=False)
    cm_d = nc.declare_dram_parameter("cmask", [PB, 256], BF16, isOutput=False)
    id_d = nc.declare_dram_parameter("ident", [E, E], BF16, isOutput=False)
    out_d = nc.declare_dram_parameter("out", [E + 1, S], F32, isOutput=True)

    with tile.TileContext(nc) as tc:
        with tc.tile_pool(name="persist", bufs=1) as pp, \
             tc.tile_pool(name="work", bufs=6) as wp, \
             tc.tile_pool(name="osb", bufs=3) as op:
            # ---- persistent SBUF tiles ----
            wq8_sb = pp.tile([PB, 256], F8, name="wq8_sb", tag="wq8_sb")
            wk8_sb = pp.tile([PB, 256], F8, name="wk8_sb", tag="wk8_sb")
            wv_sb = pp.tile([PB, ND * E], BF16, name="wv_sb", tag="wv_sb")
            mk_sb = pp.tile([PB, 256], BF16, name="mk_sb", tag="mk_sb")
            idb_sb = pp.tile([E, E], BF16, name="idb_sb", tag="idb_sb")
            xq8_sb = pp.tile([PB, 16384], F8, name="xq8_sb", tag="xq8_sb")
            xk8_sb = pp.tile([PB, 8192], F8, name="xk8_sb", tag="xk8_sb")
            xv_sb = [pp.tile([PB, 2048], BF16, name=f"xv{d}", tag=f"xv{d}")
                     for d in range(ND)]
            qpT8 = pp.tile([32, 2 * S], F8, name="qpT8", tag="qpT8")
            kpT8 = pp.tile([32, S], F8, name="kpT8", tag="kpT8")
            vpT = pp.tile([E, S // 2], BF16, name="vpT", tag="vpT")
            vp = pp.tile([PB, NL * (E + 1)], BF16, name="vp", tag="vp")

            # plane views
            xq8v = xq8_sb[:].rearrange("p (H g s j) -> p H g s j", H=2, g=2, s=2)
            xk8v = xk8_sb[:].rearrange("p (g s j) -> p g s j", g=2, s=2)
            wq8v = wq8_sb[:].rearrange("p (g s m e) -> p g s m e", g=2, s=2, m=2)
            wk8v = wk8_sb[:].rearrange("p (g s m e) -> p g s m e", g=2, s=2, m=2)
            qpT8v = qpT8[:].rearrange("p (s j) -> p s j", s=2)
            kpT8v = kpT8[:].rearrange("p (s j) -> p s j", s=2)
            vpv = vp[:].rearrange("p (l e) -> p l e", e=E + 1)

            # ---- DMAs (two queues: sync + gpsimd) ----
            nc.gpsimd.dma_start(out=wk8_sb[:], in_=wk8_d[:])
            nc.gpsimd.dma_start(out=wq8_sb[:], in_=wq8_d[:])
            nc.gpsimd.dma_start(
                out=wv_sb[:].rearrange("p (d e) -> p d e", e=E),
                in_=wv_d.rearrange("(d p) e -> p d e", p=PB))
            nc.gpsimd.dma_start(out=mk_sb[:], in_=cm_d[:])
            nc.gpsimd.dma_start(out=idb_sb[:], in_=id_d[:])
            nc.sync.dma_start(out=xk8_sb[:, 0:4096], in_=xk8_d[:, 0:4096])
            nc.gpsimd.dma_start(out=xk8_sb[:, 4096:8192], in_=xk8_d[:, 4096:8192])
            # xq half 0 (first 8192 cols), split across queues
            nc.sync.dma_start(out=xq8_sb[:, 0:4096], in_=xq8_d[:, 0:4096])
            nc.gpsimd.dma_start(out=xq8_sb[:, 4096:8192], in_=xq8_d[:, 4096:8192])
            for d in range(ND):
                nc.sync.dma_start(out=xv_sb[d][:], in_=xv_d[PB * d:PB * (d + 1), :])
            nc.gpsimd.dma_start(out=xq8_sb[:, 8192:12288], in_=xq8_d[:, 8192:12288])
            nc.gpsimd.dma_start(out=xq8_sb[:, 12288:16384], in_=xq8_d[:, 12288:16384])

            # ones column of vp
            nc.vector.memset(vpv[:, :, E:E + 1], 1.0)

            # ---- prologue: projections ----
            copy_eng = [nc.vector, nc.scalar]
            with tc.tile_pool(name="pj8_ps", bufs=2, space="PSUM") as pj8p, \
                 tc.tile_pool(name="pjv_ps", bufs=2, space="PSUM") as pjvp, \
                 tc.tile_pool(name="vt_ps", bufs=2, space="PSUM") as vtp:
                def proj8(w8v, x8gsj, dstv, dst_off, ci):
                    """One 512-col fp8 DR piece -> both E-half planes."""
                    pj = pj8p.tile([32, 1024], F32, name=f"pj8_{dstv is qpT8v}_{dst_off}", tag="pj8")
                    for m in range(2):
                        for g in range(2):
                            nc.tensor.matmul(
                                pj[:, 512 * m:512 * (m + 1)],
                                w8v[:, g, :, m, :],
                                x8gsj(g),
                                start=(g == 0), stop=(g == 1), perf_mode=DR)
                    dst_ap = dstv[0:32, :, dst_off:dst_off + 512]
                    src_ap = pj[:].rearrange("p (s j) -> p s j", s=2)
                    if copy_eng[ci % 2] is nc.scalar:
                        nc.scalar.copy(dst_ap, src_ap)
                    else:
                        nc.vector.tensor_copy(dst_ap, src_ap)

                ci = 0
                for i in range(4):      # K: parity half, 2048 cols
                    proj8(wk8v, lambda g, i=i: xk8v[:, g, :, 512 * i:512 * (i + 1)],
                          kpT8v, 512 * i, ci)
                    ci += 1
                for j in range(4):      # Q half 0
                    proj8(wq8v, lambda g, j=j: xq8v[:, 0, g, :, 512 * j:512 * (j + 1)],
                          qpT8v, 512 * j, ci)
                    ci += 1

                def projv(i):           # V: bf16 piece
                    pj = pjvp.tile([E, 512], F32, name=f"pjv{i}", tag="pjv")
                    for d in range(ND):
                        nc.tensor.matmul(pj[:], wv_sb[:, E * d:E * (d + 1)],
                                         xv_sb[d][:, 512 * i:512 * (i + 1)],
                                         start=(d == 0), stop=(d == ND - 1))
                    nc.vector.tensor_copy(vpT[:, 512 * i:512 * (i + 1)], pj[:])

                for i in range(4):
                    projv(i)
                # V -> k-major vp blocks (PE transpose, batched via PSUM)
                for t in range(2):
                    vt = vtp.tile([PB, 8 * E], BF16, name=f"vt{t}", tag="vt")
                    for j in range(8):
                        l = 8 * t + j
                        nc.tensor.transpose(vt[:, E * j:E * (j + 1)],
                                            vpT[:, PB * l:PB * (l + 1)],
                                            idb_sb[:])
                    nc.vector.tensor_copy(vpv[:, 8 * t:8 * t + 8, 0:E],
                                          vt[:].rearrange("p (l e) -> p l e", e=E))
                for j in range(4, 8):   # Q half 1
                    proj8(wq8v, lambda g, j=j: xq8v[:, 1, g, :, 512 * (j - 4):512 * (j - 3)],
                          qpT8v, 512 * j, ci)
                    ci += 1

            # ---- attention ----
            with tc.tile_pool(name="st_ps", bufs=2, space="PSUM") as stp, \
                 tc.tile_pool(name="zt_ps", bufs=1, space="PSUM") as ztp:
                for (q0, W, Lmax) in CHUNKS:
                    zt = ztp.tile([E + 1, 2048], F32, name=f"zt{q0}", tag="zt")
                    pend = []

                    def drain_av(l, o, ets, q0=q0, W=W, Lmax=Lmax, zt=zt):
                        for (a, b) in st_segs(o, W):
                            et, u = ets[a // 1024]
                            nc.tensor.matmul(
                                zt[:, a:b], vp[:, (E + 1) * l:(E + 1) * (l + 1)],
                                et[:, a - 1024 * u:b - 1024 * u],
                                start=(l == 0),
                                stop=(l == l_last(q0, a // 512, Lmax)),
                                skip_group_check=True)
                        # strips whose accumulation just completed -> out
                        for s in range(W // 512):
                            if l_last(q0, s, Lmax) == l:
                                zs = op.tile([E + 1, 512], F32, name=f"zs{q0}_{s}", tag="zs")
                                nc.vector.tensor_copy(zs[:], zt[:, 512 * s:512 * (s + 1)])
                                nc.gpsimd.dma_start(
                                    out=out_d[:, q0 + 512 * s:q0 + 512 * (s + 1)],
                                    in_=zs[:])

                    for l in range(Lmax + 1):
                        o = max(0, 256 * l - q0)
                        segs = st_segs(o, W)
                        st_tiles = {}
                        for u in sorted({a // 1024 for (a, _) in segs}):
                            st_tiles[u] = stp.tile([PB, 1024], F32,
                                                   name=f"st{q0}_{l}_{u}", tag="st")
                        for (a, b) in segs:
                            u = a // 1024
                            nc.tensor.matmul(
                                st_tiles[u][:, a - 1024 * u:b - 1024 * u],
                                kpT8v[0:32, :, PB * l:PB * (l + 1)],
                                qpT8v[0:32, :, q0 + a:q0 + b],
                                start=True, stop=True, perf_mode=DR)
                        ets = {}
                        for u, stt in st_tiles.items():
                            et = wp.tile([PB, 1024], BF16, name=f"et{q0}_{l}_{u}", tag="et")
                            aw = max(o - 1024 * u, 0)
                            ew = min(W - 1024 * u, 1024)
                            nc.scalar.activation(
                                et[:, aw:ew], stt[:, aw:ew],
                                mybir.ActivationFunctionType.Exp,
                                scale=0.125 / (WSC * WSC))
                            ets[u] = (et, u)
                        if q0 <= 256 * l < q0 + W:   # diagonal in this chunk
                            u = o // 1024
                            mo = o - 1024 * u
                            et = ets[u][0]
                            nc.vector.tensor_mul(et[:, mo:mo + 256],
                                                 et[:, mo:mo + 256], mk_sb[:])
                        pend.append((l, o, ets))
                        if len(pend) > LAG:
                            drain_av(*pend.pop(0))
                    while pend:
                        drain_av(*pend.pop(0))
    nc.finalize()
    return nc


def make_core_inputs(key_np, value_np, query_np, Wk, Wv, Wq):
    """Host-side sharding: returns in_maps list of 8 dicts."""
    bf = lambda a: np.ascontiguousarray(a).astype(NPBF16)
    f8 = lambda a: np.ascontiguousarray(a).astype(NPF8)
    ki = np.arange(PB)[:, None]
    qi = np.arange(PB)[None, :]
    tri = (ki <= qi).astype(np.float32)
    ones = np.ones((PB, PB), np.float32)
    zeros = np.zeros((PB, PB), np.float32)

    def pack_w(Wm):  # [512, 64] -> [p, (g,s,m,e)] x WSC
        a = (Wm * WSC).reshape(2, 2, PB, 2, 32)          # (g, s, p, m, e)
        return f8(a.transpose(2, 0, 1, 3, 4).reshape(PB, 256))

    def pack_xq(Xq):  # [4096, 512] -> [p, (H,g,s,j)]
        a = Xq.T.reshape(2, 2, PB, 2, 2048)              # (g, s, p, H, j)
        return f8(a.transpose(2, 3, 0, 1, 4).reshape(PB, 16384))

    def pack_xk(XkT):  # [512, 2048] -> [p, (g,s,j)]
        a = XkT.reshape(2, 2, PB, 2048)                  # (g, s, p, j)
        return f8(a.transpose(2, 0, 1, 3).reshape(PB, 8192))

    in_maps = []
    for c in range(8):
        b, p = c // 2, c % 2
        kcols = np.concatenate(
            [np.arange(PB * (2 * l + p), PB * (2 * l + p) + PB) for l in range(NL)])
        cmask = np.concatenate([tri, ones] if p == 0 else [zeros, tri], axis=1)
        in_maps.append({
            "xq8": pack_xq(query_np[b]),
            "xk8": pack_xk(key_np[b].T[:, kcols]),
            "wq8": pack_w(Wq), "wk8": pack_w(Wk),
            "xv": bf(value_np[b].T[:, kcols]),
            "wv": bf(Wv),
            "cmask": bf(cmask),
            "ident": bf(np.eye(E, dtype=np.float32)),
        })
    return in_maps


def assemble_output(results):
    """results: 8 dicts with 'out' [65, S] f32 partials -> Z [B,S,E]."""
    Z = np.zeros((B, S, E), dtype=np.float32)
    for b in range(B):
        A = results[2 * b]["out"].astype(np.float32) + \
            results[2 * b + 1]["out"].astype(np.float32)
        Z[b] = (A[:E] / A[E:E + 1]).T
    return Z


def kernel(key_inputs, value_inputs, query_inputs, Wk, Wv, Wq):
    from concourse.bass_utils import run_bass_kernel_spmd
    nc = build_nc()
    in_maps = make_core_inputs(np.asarray(key_inputs), np.asarray(value_inputs),
                               np.asarray(query_inputs), np.asarray(Wk),
                               np.asarray(Wv), np.asarray(Wq))
    res = run_bass_kernel_spmd(nc, in_maps, core_ids=list(range(8)))
    return assemble_output(res.results)


# revision 8
# speedup vs baseline: 1.0915x; 1.0678x over previous
"""Distributed causal attention head on 8 TRN2 NeuronCores.

v3: parity-split sharding + fp8 DoubleRow score path.

Sharding: core c = 2*b + p handles batch b and the KEY/VALUE blocks of
parity p (global 128-row k-blocks {2l+p}).  Each core computes partial
attention numerators Z_p^T = V_p^T P_p and partial denominators over its
k-parity for ALL q of the batch; the host sums the two partials per
batch and normalizes (flash-attention partial-softmax combine; no
max-subtraction needed since |scores/8| < ~1.5).

The Q/K path runs in fp8e4m3 with DoubleRow matmuls (2 contraction
subtiles per pass, 0.5 cycles/col): X_q/X_k arrive fp8 plane-packed,
W_q/W_k arrive fp8 pre-scaled by 32 (folded out in the exp scale), the
projections emit Q^T/K^T as [32, 2-plane] fp8, and score matmuls
contract E=64 as 2x32 planes.  The V/AV path stays bf16 for accuracy
(fp8 probs/values fail the 2e-2 gate).  PSUM is always f32.

Schedule is SPMD-identical across cores; parity lives in the DATA
(xk/xv contents and one [128,256] causal mask).  Score matmuls stream
only the causal-valid q-suffix (offset 256*l = max over parities); the
one partially-masked 256-col region per k-block is masked on DVE after
the ACT exp.  AV accumulates zt[65, q] in PSUM banks with start/stop
per 512-col bank; finished strips are copied out and DMA'd as soon as
their last k-block lands.  Projections run in a PSUM-pool prologue
(copies alternate DVE/ACT to keep pace with the PE).
"""

import numpy as np
import ml_dtypes

import concourse.bass as bass
import concourse.bacc as bacc
import concourse.mybir as mybir
import concourse.tile as tile

B, S, D, E = 4, 4096, 512, 64
PB = 128                     # partition block
NL = 16                      # local k-blocks per core (parity half)
ND = 4                       # d-slices
LAG = 2                      # ST -> AV pipeline depth (in l's)
WSC = 32.0                   # fp8 weight pre-scale (host); folded into exp
# attention q-chunks: (q0, width, Lmax)
CHUNKS = [(0, 2048, 7), (2048, 1024, 11), (3072, 1024, 15)]
BF16 = mybir.dt.bfloat16
F32 = mybir.dt.float32
F8 = mybir.dt.float8e4
NPBF16 = ml_dtypes.bfloat16
NPF8 = ml_dtypes.float8_e4m3
DR = mybir.MatmulPerfMode.DoubleRow


def st_segs(o, W):
    """512-bank-aligned segments covering [o, W)."""
    segs, a = [], o
    while a < W:
        b = min((a // 512 + 1) * 512, W)
        segs.append((a, b))
        a = b
    return segs


def l_last(q0, s, Lmax):
    """Last local k-block whose stream covers 512-col strip s of chunk."""
    return min(Lmax, (q0 + 512 * s + 511) // 256)


def build_nc():
    nc = bacc.Bacc(None)

    # fp8 Q/K inputs, plane-packed on host:
    #   xq8[p, (H,g,s,j)]: H = q-col half (2048), g = d-pair, s = d-slice in pair
    #   xk8[p, (g,s,j)]  : j over the 2048 parity-packed k cols
    #   wq8/wk8[p, (g,s,m,e)]: m = E-half (out plane), e in 0..31, pre-scaled x32
    xq8_d = nc.declare_dram_parameter("xq8", [PB, 16384], F8, isOutput=False)
    xk8_d = nc.declare_dram_parameter("xk8", [PB, 8192], F8, isOutput=False)
    wq8_d = nc.declare_dram_parameter("wq8", [PB, 256], F8, isOutput=False)
    wk8_d = nc.declare_dram_parameter("wk8", [PB, 256], F8, isOutput=False)
    xv_d = nc.declare_dram_parameter("xv", [D, S // 2], BF16, isOutput=False)
    wv_d = nc.declare_dram_parameter("wv", [D, E], BF16, isOutput=False)
    cm_d = nc.declare_dram_parameter("cmask", [PB, 256], BF16, isOutput=False)
    id_d = nc.declare_dram_parameter("ident", [E, E], BF16, isOutput=False)
    out_d = nc.declare_dram_parameter("out", [E + 1, S], F32, isOutput=True)

    with tile.TileContext(nc) as tc:
        with tc.tile_pool(name="persist", bufs=1) as pp, \
             tc.tile_pool(name="work", bufs=6) as wp, \
             tc.tile_pool(name="osb", bufs=3) as op:
            # ---- persistent SBUF tiles ----
            wq8_sb = pp.tile([PB, 256], F8, name="wq8_sb", tag="wq8_sb")
            wk8_sb = pp.tile([PB, 256], F8, name="wk8_sb", tag="wk8_sb")
            wv_sb = pp.tile([PB, ND * E], BF16, name="wv_sb", tag="wv_sb")
            mk_sb = pp.tile([PB, 256], BF16, name="mk_sb", tag="mk_sb")
            idb_sb = pp.tile([E, E], BF16, name="idb_sb", tag="idb_sb")
            xq8_sb = pp.tile([PB, 16384], F8, name="xq8_sb", tag="xq8_sb")
            xk8_sb = pp.tile([PB, 8192], F8, name="xk8_sb", tag="xk8_sb")
            xv_sb = [pp.tile([PB, 2048], BF16, name=f"xv{d}", tag=f"xv{d}")
                     for d in range(ND)]
            qpT = pp.tile([E, S], BF16, name="qpT", tag="qpT")
            kpT = pp.tile([E, S // 2], BF16, name="kpT", tag="kpT")
            vpT = pp.tile([E, S // 2], BF16, name="vpT", tag="vpT")
            vp = pp.tile([PB, NL * (E + 1)], BF16, name="vp", tag="vp")

            # plane views
            xq8v = xq8_sb[:].rearrange("p (H g s j) -> p H g s j", H=2, g=2, s=2)
            xk8v = xk8_sb[:].rearrange("p (g s j) -> p g s j", g=2, s=2)
            wq8v = wq8_sb[:].rearrange("p (g s m e) -> p g s m e", g=2, s=2, m=2)
            wk8v = wk8_sb[:].rearrange("p (g s m e) -> p g s m e", g=2, s=2, m=2)
            vpv = vp[:].rearrange("p (l e) -> p l e", e=E + 1)

            # ---- DMAs (two queues: sync + gpsimd) ----
            nc.gpsimd.dma_start(out=wk8_sb[:], in_=wk8_d[:])
            nc.gpsimd.dma_start(out=wq8_sb[:], in_=wq8_d[:])
            nc.gpsimd.dma_start(
                out=wv_sb[:].rearrange("p (d e) -> p d e", e=E),
                in_=wv_d.rearrange("(d p) e -> p d e", p=PB))
            nc.gpsimd.dma_start(out=mk_sb[:], in_=cm_d[:])
            nc.gpsimd.dma_start(out=idb_sb[:], in_=id_d[:])
            nc.sync.dma_start(out=xk8_sb[:, 0:4096], in_=xk8_d[:, 0:4096])
            nc.gpsimd.dma_start(out=xk8_sb[:, 4096:8192], in_=xk8_d[:, 4096:8192])
            # xq half 0 (first 8192 cols), split across queues
            nc.sync.dma_start(out=xq8_sb[:, 0:4096], in_=xq8_d[:, 0:4096])
            nc.gpsimd.dma_start(out=xq8_sb[:, 4096:8192], in_=xq8_d[:, 4096:8192])
            for d in range(ND):
                nc.sync.dma_start(out=xv_sb[d][:], in_=xv_d[PB * d:PB * (d + 1), :])
            nc.gpsimd.dma_start(out=xq8_sb[:, 8192:12288], in_=xq8_d[:, 8192:12288])
            nc.gpsimd.dma_start(out=xq8_sb[:, 12288:16384], in_=xq8_d[:, 12288:16384])

            # ones column of vp
            nc.vector.memset(vpv[:, :, E:E + 1], 1.0)

            # ---- prologue: projections ----
            copy_eng = [nc.vector, nc.scalar]
            with tc.tile_pool(name="pj8_ps", bufs=2, space="PSUM") as pj8p, \
                 tc.tile_pool(name="pjv_ps", bufs=2, space="PSUM") as pjvp, \
                 tc.tile_pool(name="vt_ps", bufs=2, space="PSUM") as vtp:
                def proj8(w8v, x8gsj, dst, dst_off, ci):
                    """One 512-col fp8 DoubleRow piece (contraction 2x256)."""
                    pj = pj8p.tile([E, 512], F32, name=f"pj8_{dst_off}_{ci}", tag="pj8")
                    for g in range(2):
                        nc.tensor.matmul(
                            pj[:],
                            w8v[:, g].rearrange("p s m e -> p s (m e)"),
                            x8gsj(g),
                            start=(g == 0), stop=(g == 1), perf_mode=DR)
                    dst_ap = dst[:, dst_off:dst_off + 512]
                    if copy_eng[ci % 2] is nc.scalar:
                        nc.scalar.copy(dst_ap, pj[:])
                    else:
                        nc.vector.tensor_copy(dst_ap, pj[:])

                ci = 0
                for i in range(4):      # K: parity half, 2048 cols
                    proj8(wk8v, lambda g, i=i: xk8v[:, g, :, 512 * i:512 * (i + 1)],
                          kpT, 512 * i, ci)
                    ci += 1
                for j in range(4):      # Q half 0
                    proj8(wq8v, lambda g, j=j: xq8v[:, 0, g, :, 512 * j:512 * (j + 1)],
                          qpT, 512 * j, ci)
                    ci += 1

                def projv(i):           # V: bf16 piece
                    pj = pjvp.tile([E, 512], F32, name=f"pjv{i}", tag="pjv")
                    for d in range(ND):
                        nc.tensor.matmul(pj[:], wv_sb[:, E * d:E * (d + 1)],
                                         xv_sb[d][:, 512 * i:512 * (i + 1)],
                                         start=(d == 0), stop=(d == ND - 1))
                    nc.vector.tensor_copy(vpT[:, 512 * i:512 * (i + 1)], pj[:])

                for i in range(4):
                    projv(i)
                # V -> k-major vp blocks (PE transpose, batched via PSUM)
                for t in range(2):
                    vt = vtp.tile([PB, 8 * E], BF16, name=f"vt{t}", tag="vt")
                    for j in range(8):
                        l = 8 * t + j
                        nc.tensor.transpose(vt[:, E * j:E * (j + 1)],
                                            vpT[:, PB * l:PB * (l + 1)],
                                            idb_sb[:])
                    nc.vector.tensor_copy(vpv[:, 8 * t:8 * t + 8, 0:E],
                                          vt[:].rearrange("p (l e) -> p l e", e=E))
                for j in range(4, 8):   # Q half 1
                    proj8(wq8v, lambda g, j=j: xq8v[:, 1, g, :, 512 * (j - 4):512 * (j - 3)],
                          qpT, 512 * j, ci)
                    ci += 1

            # ---- attention ----
            with tc.tile_pool(name="st_ps", bufs=2, space="PSUM") as stp, \
                 tc.tile_pool(name="zt_ps", bufs=1, space="PSUM") as ztp:
                for (q0, W, Lmax) in CHUNKS:
                    zt = ztp.tile([E + 1, 2048], F32, name=f"zt{q0}", tag="zt")
                    pend = []

                    def drain_av(l, o, ets, q0=q0, W=W, Lmax=Lmax, zt=zt):
                        for (a, b) in st_segs(o, W):
                            et, u = ets[a // 1024]
                            nc.tensor.matmul(
                                zt[:, a:b], vp[:, (E + 1) * l:(E + 1) * (l + 1)],
                                et[:, a - 1024 * u:b - 1024 * u],
                                start=(l == 0),
                                stop=(l == l_last(q0, a // 512, Lmax)),
                                skip_group_check=True)
                        # strips whose accumulation just completed -> out
                        for s in range(W // 512):
                            if l_last(q0, s, Lmax) == l:
                                zs = op.tile([E + 1, 512], F32, name=f"zs{q0}_{s}", tag="zs")
                                nc.vector.tensor_copy(zs[:], zt[:, 512 * s:512 * (s + 1)])
                                nc.gpsimd.dma_start(
                                    out=out_d[:, q0 + 512 * s:q0 + 512 * (s + 1)],
                                    in_=zs[:])

                    for l in range(Lmax + 1):
                        o = max(0, 256 * l - q0)
                        segs = st_segs(o, W)
                        st_tiles = {}
                        for u in sorted({a // 1024 for (a, _) in segs}):
                            st_tiles[u] = stp.tile([PB, 1024], F32,
                                                   name=f"st{q0}_{l}_{u}", tag="st")
                        for (a, b) in segs:
                            u = a // 1024
                            nc.tensor.matmul(
                                st_tiles[u][:, a - 1024 * u:b - 1024 * u],
                                kpT[:, PB * l:PB * (l + 1)],
                                qpT[:, q0 + a:q0 + b],
                                start=True, stop=True)
                        ets = {}
                        for u, stt in st_tiles.items():
                            et = wp.tile([PB, 1024], BF16, name=f"et{q0}_{l}_{u}", tag="et")
                            aw = max(o - 1024 * u, 0)
                            ew = min(W - 1024 * u, 1024)
                            nc.scalar.activation(
                                et[:, aw:ew], stt[:, aw:ew],
                                mybir.ActivationFunctionType.Exp,
                                scale=0.125 / (WSC * WSC))
                            ets[u] = (et, u)
                        if q0 <= 256 * l < q0 + W:   # diagonal in this chunk
                            u = o // 1024
                            mo = o - 1024 * u
                            et = ets[u][0]
                            nc.vector.tensor_mul(et[:, mo:mo + 256],
                                                 et[:, mo:mo + 256], mk_sb[:])
                        pend.append((l, o, ets))
                        if len(pend) > LAG:
                            drain_av(*pend.pop(0))
                    while pend:
                        drain_av(*pend.pop(0))
    nc.finalize()
    return nc


def make_core_inputs(key_np, value_np, query_np, Wk, Wv, Wq):
    """Host-side sharding: returns in_maps list of 8 dicts."""
    bf = lambda a: np.ascontiguousarray(a).astype(NPBF16)
    f8 = lambda a: np.ascontiguousarray(a).astype(NPF8)
    ki = np.arange(PB)[:, None]
    qi = np.arange(PB)[None, :]
    tri = (ki <= qi).astype(np.float32)
    ones = np.ones((PB, PB), np.float32)
    zeros = np.zeros((PB, PB), np.float32)

    def pack_w(Wm):  # [512, 64] -> [p, (g,s,m,e)] x WSC
        a = (Wm * WSC).reshape(2, 2, PB, 2, 32)          # (g, s, p, m, e)
        return f8(a.transpose(2, 0, 1, 3, 4).reshape(PB, 256))

    def pack_xq(Xq):  # [4096, 512] -> [p, (H,g,s,j)]
        a = Xq.T.reshape(2, 2, PB, 2, 2048)              # (g, s, p, H, j)
        return f8(a.transpose(2, 3, 0, 1, 4).reshape(PB, 16384))

    def pack_xk(XkT):  # [512, 2048] -> [p, (g,s,j)]
        a = XkT.reshape(2, 2, PB, 2048)                  # (g, s, p, j)
        return f8(a.transpose(2, 0, 1, 3).reshape(PB, 8192))

    in_maps = []
    for c in range(8):
        b, p = c // 2, c % 2
        kcols = np.concatenate(
            [np.arange(PB * (2 * l + p), PB * (2 * l + p) + PB) for l in range(NL)])
        cmask = np.concatenate([tri, ones] if p == 0 else [zeros, tri], axis=1)
        in_maps.append({
            "xq8": pack_xq(query_np[b]),
            "xk8": pack_xk(key_np[b].T[:, kcols]),
            "wq8": pack_w(Wq), "wk8": pack_w(Wk),
            "xv": bf(value_np[b].T[:, kcols]),
            "wv": bf(Wv),
            "cmask": bf(cmask),
            "ident": bf(np.eye(E, dtype=np.float32)),
        })
    return in_maps


def assemble_output(results):
    """results: 8 dicts with 'out' [65, S] f32 partials -> Z [B,S,E]."""
    Z = np.zeros((B, S, E), dtype=np.float32)
    for b in range(B):
        A = results[2 * b]["out"].astype(np.float32) + \
            results[2 * b + 1]["out"].astype(np.float32)
        Z[b] = (A[:E] / A[E:E + 1]).T
    return Z


def kernel(key_inputs, value_inputs, query_inputs, Wk, Wv, Wq):
    from concourse.bass_utils import run_bass_kernel_spmd
    nc = build_nc()
    in_maps = make_core_inputs(np.asarray(key_inputs), np.asarray(value_inputs),
                               np.asarray(query_inputs), np.asarray(Wk),
                               np.asarray(Wv), np.asarray(Wq))
    res = run_bass_kernel_spmd(nc, in_maps, core_ids=list(range(8)))
    return assemble_output(res.results)
